# revision 42
# baseline (speedup 1.0000x reference)
"""Trainium2 Bass kernel for nn_Beta_LR_41308995453190.

Network (per (b, o) pair):
  - 13 segment means over the L axis of hidden[b, o] (ragged boundaries
    from idx[b]): 10 context segments, question, option, whole-context.
  - beta-param projection e = 1 + relu(x @ Wp + bp), split a/b.
  - three attention pools (intersection over segments, renew over
    (segment, intersection) pairs, union over inverted renewed params).
  - classify head: concat 8 beta embeddings -> relu(@Wl0 + bl0) -> @Wl + bl.

Sharding: data-parallel over the batch dim B=8 (one batch per NeuronCore),
weights replicated.

Implementation notes:
  - hidden + segment masks + Wp/Wa0/Wa travel in fp8 e4m3 (measured
    end-to-end error ~3.3e-3 vs the 2e-2 gate); Wl0 stays fp16 (fp8
    there costs 2.9e-2). Activations and everything else 16-bit is fp16.
  - hidden uses a p-outer SBUF layout (partition p holds L rows
    [p*T, (p+1)*T)) so every DMA descriptor is an 8 KB contiguous run;
    option-0 hidden is the first DMA so the serial chain starts ASAP,
    small constants and weights queue behind the hidden options.
  - segment sums: per-option 0/1-mask matmuls (13-column fp8 stationary)
    packed into one [128, E] PSUM tile via PE column tile positions
    {0, 32, 64, 96}; a host-built selection matrix D (D[o*32+k, o*13+k]
    = 1/cnt_k) then transposes to feature-major while absorbing the
    1/count scaling and the 32->13 packing.
  - All layer matmuls run "flipped": the small activation block is the
    stationary operand, the weight matrix streams 512 columns at a time;
    outputs transpose back to feature-major (fp16 transposes) and the
    per-chunk biases apply in one batched vector op per layer.
  - The pair ("renew") softmax is computed as an exp blend
    1/na = (ea+1)*recip(ea*a+ia), ea = exp(l1-l2), keeping the scalar
    activation table on Exp (a Sigmoid would force two table reloads);
    the b-half elementwise work runs on gpsimd in parallel, and the
    reciprocals run per 2-chunk slice so the h3 matmuls pipeline behind
    them.
  - classify head: 33 chunks (chunk 32 = ones x bl0/128 applies the
    bias); wl0 DMA-groups and the 8..32 matmuls are ordered so they
    stream during the softmax vector phases; chunks 0..7 close the
    accumulation chain after the union.
"""

import numpy as np
import ml_dtypes

try:
    import concourse.bass as bass
except ImportError:
    import sys

    sys.path.insert(0, "/opt/trn_rl_repo")
    import concourse.bass as bass

import concourse.tile as tile
from concourse import mybir
from concourse.bass_utils import run_bass_kernel_spmd

F32 = mybir.dt.float32
F16 = mybir.dt.float16
FP8 = mybir.dt.float8e4
NPF16 = np.float16
NPF8 = ml_dtypes.float8_e4m3
AX = mybir.AxisListType.X
OP = mybir.AluOpType
AF = mybir.ActivationFunctionType

B, O, L, E = 8, 4, 1024, 1024
BETA = 512
NSEG = 12
NK = 13  # 10 ctx + q + o + allc
P = 128
T = L // P  # 8 L-tiles per option
NCOL = O * NK  # 52


# ---------------------------------------------------------------------------
# Workaround: this neuronxcc walrus build rejects more than one sem wait per
# TPB instruction ("Too many sync wait commands"). Hoist excess waits onto
# nop instructions inserted immediately before the offending instruction on
# the same engine.
# ---------------------------------------------------------------------------
def _split_excess_waits(nc, max_waits=1):
    scratch_bb = nc.cur_bb.bb
    for f in nc.m.functions:
        for bb in f.blocks:
            new_list = []
            for ins in bb.instructions:
                si = ins.sync_info
                waits = list(si.on_wait) if si and si.on_wait else []
                if len(waits) > max_waits:
                    for w in waits[: len(waits) - max_waits]:
                        carrier = nc.engines[ins.engine].nop(nofuse=True).ins
                        scratch_bb.instructions.remove(carrier)
                        carrier.sync_info = mybir.SyncInfo(
                            on_wait=[w], on_update=[]
                        )
                        new_list.append(carrier)
                    si.on_wait = waits[len(waits) - max_waits :]
                new_list.append(ins)
            bb.instructions[:] = new_list


def _patch_minimal_drain():
    """One-shot NEFF: skip the semaphore-clear + second all-engine barrier
    of the TileContext epilogue (they only matter when the program loops)."""
    from concourse.vector_clock import ScopedClock
    import concourse.tile as _tile

    def _drain_and_barrier(self, tick_clock, wait_clock):
        drain_inst = self.nc.sync.drain()
        wait_clock.add_sem_waits(
            drain_inst.ins, ScopedClock({None: tick_clock.global_clock})
        )
        self.nc.all_engine_barrier()
        assert self.sems is not None
        popped = self.nc._tile_sem_poison_stack.pop()
        assert popped is self._sem_poison

    _tile.TileContext._drain_and_barrier = _drain_and_barrier


_patch_minimal_drain()


def _build_nc(debug=False):
    nc = bass.Bass("TRN2", target_bir_lowering=False)

    hid_d = nc.dram_tensor("hidden", [O, L, E], FP8, kind="ExternalInput")
    mask_d = nc.dram_tensor("maskt", [P, T, NK], FP8, kind="ExternalInput")
    dmat_d = nc.dram_tensor("dmat", [P, NCOL], F16, kind="ExternalInput")
    ident_d = nc.dram_tensor("ident", [P, P], F16, kind="ExternalInput")
    wp_d = nc.dram_tensor("wp", [P, 8, 1024], FP8, kind="ExternalInput")
    wa0_d = nc.dram_tensor("wa0", [P, 8, 512], FP8, kind="ExternalInput")
    wa_d = nc.dram_tensor("wa", [P, 4, 512], FP8, kind="ExternalInput")
    # 33 chunks: chunk 32 is bl0/128 replicated, matched by an all-ones lhsT
    # column so the head matmul chain also applies the bl0 bias
    wl0_d = nc.dram_tensor("wl0", [P, 33, 512], F16, kind="ExternalInput")
    bias_d = nc.dram_tensor("biases", [P, 21], F32, kind="ExternalInput")
    wlr_d = nc.dram_tensor("wlrep", [O, 512], F32, kind="ExternalInput")
    out_d = nc.dram_tensor("out", [O, 1], F32, kind="ExternalOutput")

    with tile.TileContext(nc) as tc:
        with (
            tc.tile_pool(name="const", bufs=1) as const,
            tc.tile_pool(name="hidp2", bufs=4) as hidp2,
            tc.tile_pool(name="act", bufs=1) as act,
            tc.tile_pool(name="tmp", bufs=3) as tmp,
            tc.tile_pool(name="rows", bufs=1) as rowsp,
            tc.tile_pool(name="pseg", bufs=1, space="PSUM") as pseg,
            tc.tile_pool(name="prow", bufs=2, space="PSUM") as prow,
            tc.tile_pool(name="pt", bufs=2, space="PSUM") as pt,
        ):
            # ---- option-0 hidden goes absolutely first: the whole serial
            # chain (seg -> x -> e -> ... -> head) keys off hidden arrival
            htile0 = hidp2.tile([P, T, E], FP8, tag="htile")
            hid_r = hid_d.rearrange("o (p t) e -> o p t e", t=T)
            for q in range(4):
                nc.sync.dma_start(
                    out=htile0[q * 32 : q * 32 + 32, :, :],
                    in_=hid_r[0][q * 32 : q * 32 + 32, :, :],
                )

            # ---- mask first (needed by the first seg matmul); the other
            # small constants queue after the hidden options so the hidden
            # descriptor writes aren't delayed
            mask_sb = const.tile([P, T, NK], FP8)
            nc.sync.dma_start(out=mask_sb, in_=mask_d[:])
            dmat_sb = const.tile([P, NCOL], F16)
            ident = const.tile([P, P], F16)
            bias_sb = const.tile([P, 21], F32)
            wlr_sb = const.tile([O, 512], F32)

            # ---- packed segment sums: ps_all[o*32+k, e], one 13-column
            # fp8 mask stationary per option at PE column position o*32, so
            # each option's sums touch only its own partitions (this keeps
            # the two option-pair chains below independent).
            ps_all = pseg.tile([P, E], F32)
            nc.vector.memset(ps_all, 0.0)

            wp_sb = wa0_sb = wa_sb = wl0_sb = None
            for o in range(O):
                if o == 0:
                    htile = htile0
                else:
                    htile = hidp2.tile([P, T, E], FP8, tag="htile")
                    for q in range(4):
                        nc.sync.dma_start(
                            out=htile[q * 32 : q * 32 + 32, :, :],
                            in_=hid_r[o][q * 32 : q * 32 + 32, :, :],
                        )
                if o == O - 1:
                    # remaining small constants, then weights (fp8 except
                    # wl0); wl0 chunk groups ordered 8..32 first (those
                    # head matmuls only need eT), 0..7 (union) last
                    nc.sync.dma_start(out=dmat_sb, in_=dmat_d[:])
                    nc.sync.dma_start(out=ident, in_=ident_d[:])
                    nc.sync.dma_start(out=bias_sb, in_=bias_d[:])
                    nc.sync.dma_start(out=wlr_sb, in_=wlr_d[:])
                    # preload the scalar activation table off the critical
                    # path (the first scalar op otherwise loads it mid-kernel)
                    actwarm = const.tile([1, 1], F32)
                    nc.scalar.activation(
                        out=actwarm, in_=bias_sb[0:1, 0:1], func=AF.Exp
                    )
                    wp_sb = const.tile([P, 8, 1024], FP8)
                    for q in range(2):
                        nc.sync.dma_start(
                            out=wp_sb[:, q * 4 : q * 4 + 4, :],
                            in_=wp_d[:, q * 4 : q * 4 + 4, :],
                        )
                    wa0_sb = const.tile([P, 8, 512], FP8)
                    nc.sync.dma_start(out=wa0_sb, in_=wa0_d[:])
                    wa_sb = const.tile([P, 4, 512], FP8)
                    nc.sync.dma_start(out=wa_sb, in_=wa_d[:])
                    wl0_sb = const.tile([P, 33, 512], F16)
                    for g, w in ((8, 4), (12, 4), (16, 4), (20, 4), (24, 4),
                                 (28, 5), (0, 4), (4, 4)):
                        nc.sync.dma_start(
                            out=wl0_sb[:, g : g + w, :],
                            in_=wl0_d[:, g : g + w, :],
                        )
                for half in range(2):
                    sl = slice(half * 512, half * 512 + 512)
                    for t in range(T):
                        nc.tensor.matmul(
                            out=ps_all[o * 32 : o * 32 + NK, sl],
                            lhsT=mask_sb[:, t, :],
                            rhs=htile[:, t, sl],
                            start=(t == 0),
                            stop=(t == T - 1),
                            tile_position=(0, o * 32),
                        )

            # ---- the rest of the network runs as two independent
            # option-pair chains: chain 0 (options 0,1) starts as soon as
            # their hidden arrives, overlapping options 2,3's DMA; chain 1's
            # matmuls then fill chain 0's vector-phase PE gaps.
            x_sb = rowsp.tile([P, E], F16, tag="x_sb")
            xT = act.tile([P, 8, O, NK], F16)
            eT16 = act.tile([P, 8, O, NK], F16)
            catF16 = act.tile([P, 33, O], F16)
            nc.gpsimd.memset(catF16[:, 32, :], 1.0)
            cat2 = act.tile([P, 8, O], F32)
            cat2b = act.tile([P, 8, O], F16)
            l1T = act.tile([P, 4, O, NK], F32)
            l2T = act.tile([P, 4, O], F32)
            raT = act.tile([P, 4, O, 10], F16)
            rbT = act.tile([P, 4, O, 10], F16)
            l3T = act.tile([P, 4, O, 10], F32)
            catFu = act.tile([P, 8, O], F32)

            R = O * NK  # 52 rows

            def flip_layer(h, name, lhs_chunks, w_sb, n_out, r):
                """out rows = (lhs^T)^T @ W; returns the row-major fp16
                sbuf copy [r, n_out] for this chain."""
                rows_full = rowsp.tile([R, 1024], F16, tag=f"rows{h}")
                rows_sb = rows_full[:r, :n_out]
                for n2 in range(n_out // 512):
                    pr = prow.tile([r, 512], F32, tag="prow")
                    for c, lhs in enumerate(lhs_chunks):
                        nc.tensor.matmul(
                            out=pr,
                            lhsT=lhs,
                            rhs=w_sb[:, c, n2 * 512 : (n2 + 1) * 512]
                            if w_sb.shape[2] > 512
                            else w_sb[:, c, :],
                            start=(c == 0),
                            stop=(c == len(lhs_chunks) - 1),
                        )
                    b = n2 * 512
                    nc.scalar.copy(
                        out=rows_sb[:, b : b + 256], in_=pr[:, 0:256]
                    )
                    nc.vector.tensor_copy(
                        out=rows_sb[:, b + 256 : b + 512], in_=pr[:, 256:512]
                    )
                return rows_sb

            def transpose_all(rows_sb, r, n_out):
                tpt = pt.tile([P, n_out // P, r], F16, tag="ptT")
                for mc in range(n_out // P):
                    nc.tensor.transpose(
                        out=tpt[:, mc, :],
                        in_=rows_sb[:, mc * P : (mc + 1) * P],
                        identity=ident[:r, :r],
                    )
                return tpt

            def chain(h):
                osl = slice(0, O)
                psl = slice(0, P)
                # x: fp16 copy of the packed sums per 128-column chunk
                # (alternating engines) so each scaled D-transpose starts as
                # soon as its chunk lands
                for c in range(8):
                    cs = slice(c * P, (c + 1) * P)
                    if c % 2 == 0:
                        nc.scalar.copy(out=x_sb[:, cs], in_=ps_all[:, cs])
                    else:
                        nc.vector.tensor_copy(out=x_sb[:, cs], in_=ps_all[:, cs])
                for c in range(8):
                    xtp = prow.tile([P, R], F32, tag="prow")
                    nc.tensor.matmul(
                        out=xtp,
                        lhsT=x_sb[:, c * P : (c + 1) * P],
                        rhs=dmat_sb,
                        start=True,
                        stop=True,
                    )
                    if c % 2 == 0:
                        nc.scalar.copy(out=xT[:, c, osl, :], in_=xtp)
                    else:
                        nc.vector.tensor_copy(out=xT[:, c, osl, :], in_=xtp)

                yield "x"
                # projection e = max(x @ Wp + (bp + 1), 1)
                rows_e = flip_layer(
                    h, "e", [xT[:, c, osl, :] for c in range(8)], wp_sb,
                    1024, R,
                )
                tp_e = transpose_all(rows_e, R, 1024)
                e_flat = eT16[:, :, osl, :].rearrange("p m o k -> p m (o k)")
                nc.vector.tensor_tensor(
                    out=e_flat,
                    in0=tp_e,
                    in1=bias_sb[:, 0:8].broadcast_to([P, 8, R]),
                    op=OP.add,
                )
                nc.vector.tensor_scalar_max(out=e_flat, in0=e_flat, scalar1=1.0)

                # catF chunks 8..31 only need eT
                for j, (ab, k) in enumerate(
                    ((0, 12), (1, 12), (0, 11), (1, 11), (0, 10), (1, 10))
                ):
                    nc.gpsimd.tensor_copy(
                        out=catF16[:, 8 + j * 4 : 12 + j * 4, osl],
                        in_=eT16[:, ab * 4 : ab * 4 + 4, osl, k],
                    )

                yield "e"
                # pool 1 (intersection): h1 = relu(e @ Wa0 + ba0)
                h1Tb = act.tile([P, 4, R], F16, tag=f"h1T{h}")
                rows_h1 = flip_layer(
                    h, "h1", [eT16[:, c, osl, :] for c in range(8)], wa0_sb,
                    512, R,
                )
                tp_h1 = transpose_all(rows_h1, R, 512)
                nc.vector.tensor_tensor(
                    out=h1Tb,
                    in0=tp_h1,
                    in1=bias_sb[:, 8:12].broadcast_to([P, 4, R]),
                    op=OP.add,
                )
                nc.vector.tensor_scalar_max(out=h1Tb, in0=h1Tb, scalar1=0.0)

                yield "h1"
                # l1 = h1 @ Wa + ba
                rows_l1 = flip_layer(
                    h, "l1", [h1Tb[:, c, :] for c in range(4)], wa_sb, 512, R
                )
                tp_l1 = transpose_all(rows_l1, R, 512)
                nc.vector.tensor_tensor(
                    out=l1T[:, :, osl, :].rearrange("p m o k -> p m (o k)"),
                    in0=tp_l1,
                    in1=bias_sb[:, 12:16].broadcast_to([P, 4, R]),
                    op=OP.add,
                )

                yield "l1"
                # pool 1 softmax over the 10 ctx segments + weighted reduce
                lsl = l1T[:, :, osl, 0:10]
                mx = tmp.tile([P, 4, O], F32, tag=f"mx{h}")
                nc.vector.reduce_max(mx, lsl, axis=AX)
                d = tmp.tile([P, 4, O, 10], F32, tag=f"d{h}")
                nc.vector.tensor_tensor(
                    out=d, in0=lsl, in1=mx.broadcast_to([P, 4, O, 10]),
                    op=OP.subtract,
                )
                w = tmp.tile([P, 4, O, 10], F32, tag=f"w{h}")
                nc.scalar.activation(out=w, in_=d, func=AF.Exp)
                sums = tmp.tile([P, 4, O], F32, tag=f"s{h}")
                nc.vector.reduce_sum(sums, w, axis=AX)
                r_ = tmp.tile([P, 4, O], F32, tag=f"r{h}")
                nc.vector.reciprocal(out=r_, in_=sums)
                wa_t = tmp.tile([P, 4, O, 10], F32, tag=f"wa_t{h}")
                nc.vector.tensor_tensor(
                    out=wa_t, in0=w, in1=eT16[:, 0:4, osl, 0:10], op=OP.mult
                )
                sa1 = tmp.tile([P, 4, O], F32, tag=f"sa1{h}")
                nc.vector.reduce_sum(sa1, wa_t, axis=AX)
                nc.vector.tensor_tensor(
                    out=cat2[:, 0:4, osl], in0=sa1, in1=r_, op=OP.mult
                )
                wb_t = tmp.tile([P, 4, O, 10], F32, tag=f"wb_t{h}")
                nc.gpsimd.tensor_tensor(
                    out=wb_t, in0=w, in1=eT16[:, 4:8, osl, 0:10], op=OP.mult
                )
                sb1 = tmp.tile([P, 4, O], F32, tag=f"sb1{h}")
                nc.vector.reduce_sum(sb1, wb_t, axis=AX)
                nc.gpsimd.tensor_tensor(
                    out=cat2[:, 4:8, osl], in0=sb1, in1=r_, op=OP.mult
                )
                nc.vector.tensor_copy(out=cat2b[:, :, osl], in_=cat2[:, :, osl])

                yield "sm1"
                # renew: h2/l2 for the intersection pair element
                h2Tb = act.tile([P, 4, O], F16, tag=f"h2T{h}")
                rows_h2 = flip_layer(
                    h, "h2", [cat2b[:, c, osl] for c in range(8)], wa0_sb,
                    512, O,
                )
                tp_h2 = transpose_all(rows_h2, O, 512)
                nc.vector.tensor_tensor(
                    out=h2Tb,
                    in0=tp_h2,
                    in1=bias_sb[:, 8:12].broadcast_to([P, 4, O]),
                    op=OP.add,
                )
                nc.vector.tensor_scalar_max(out=h2Tb, in0=h2Tb, scalar1=0.0)
                rows_l2 = flip_layer(
                    h, "l2", [h2Tb[:, c, :] for c in range(4)], wa_sb, 512, O
                )
                tp_l2 = transpose_all(rows_l2, O, 512)
                nc.vector.tensor_tensor(
                    out=l2T[:, :, osl],
                    in0=tp_l2,
                    in1=bias_sb[:, 12:16].broadcast_to([P, 4, O]),
                    op=OP.add,
                )

                yield "h2l2"
                # pair softmax as an exp blend: 1/na = (ea+1)*recip(ea*a+ia)
                l1s = l1T[:, :, osl, 0:10]
                l2b = l2T[:, :, osl].broadcast_to([P, 4, O, 10])
                d12 = tmp.tile([P, 4, O, 10], F32, tag=f"d12{h}")
                nc.vector.tensor_tensor(out=d12, in0=l1s, in1=l2b, op=OP.subtract)
                ea = tmp.tile([P, 4, O, 10], F32, tag=f"ea{h}")
                nc.scalar.activation(out=ea, in_=d12, func=AF.Exp)
                sp1 = tmp.tile([P, 4, O, 10], F32, tag=f"sp1{h}")
                # scalar engine: sp1 = Copy(ea * 1 + 1); the Pool engine
                # takes 2.6us for this op and blocks the chain
                nc.scalar.activation(
                    out=sp1, in_=ea, func=AF.Copy, bias=1.0
                )
                nvs = []
                for ab, dst, eng in (
                    (0, raT, nc.vector),
                    (1, rbT, nc.gpsimd),
                ):
                    ia_b = cat2[:, ab * 4 : ab * 4 + 4, osl].broadcast_to(
                        [P, 4, O, 10]
                    )
                    t1 = tmp.tile([P, 4, O, 10], F32, tag=f"t1{ab}{h}")
                    eng.tensor_tensor(
                        out=t1,
                        in0=ea,
                        in1=eT16[:, ab * 4 : ab * 4 + 4, osl, 0:10],
                        op=OP.mult,
                    )
                    nv = tmp.tile([P, 4, O, 10], F32, tag=f"nv{ab}{h}")
                    # both nv adds on gpsimd: the same add on vector measures
                    # 1.4us (vs 0.5 here), and vector is the recip bottleneck
                    nc.gpsimd.tensor_tensor(out=nv, in0=t1, in1=ia_b, op=OP.add)
                    nvs.append((dst, nv))
                with nc.allow_low_precision("1/na feeds fp16 consumers"):
                    for dst, nv in nvs:
                        for c0 in (0, 2):
                            rc = tmp.tile([P, 2, O, 10], F32, tag=f"rc{c0}{h}")
                            nc.vector.reciprocal(
                                out=rc, in_=nv[:, c0 : c0 + 2, :, :]
                            )
                            nc.vector.tensor_tensor(
                                out=dst[:, c0 : c0 + 2, osl, :],
                                in0=rc,
                                in1=sp1[:, c0 : c0 + 2, :, :],
                                op=OP.mult,
                            )

                yield "pair"
                # union pool over segments of [1/na; 1/nb]
                h3Tb = act.tile([P, 4, O, 10], F16, tag=f"h3T{h}")
                rows_h3 = flip_layer(
                    h, "h3",
                    [raT[:, c, osl, :] for c in range(4)]
                    + [rbT[:, c, osl, :] for c in range(4)],
                    wa0_sb, 512, O * 10,
                )
                tp_h3 = transpose_all(rows_h3, O * 10, 512)
                h3_flat = h3Tb.rearrange("p m o k -> p m (o k)")
                nc.vector.tensor_tensor(
                    out=h3_flat,
                    in0=tp_h3,
                    in1=bias_sb[:, 8:12].broadcast_to([P, 4, O * 10]),
                    op=OP.add,
                )
                nc.vector.tensor_scalar_max(out=h3_flat, in0=h3_flat, scalar1=0.0)
                yield "h3"
                rows_l3 = flip_layer(
                    h, "l3", [h3Tb[:, c, :, :] for c in range(4)], wa_sb,
                    512, O * 10,
                )
                tp_l3 = transpose_all(rows_l3, O * 10, 512)
                nc.vector.tensor_tensor(
                    out=l3T[:, :, osl, :].rearrange("p m o k -> p m (o k)"),
                    in0=tp_l3,
                    in1=bias_sb[:, 12:16].broadcast_to([P, 4, O * 10]),
                    op=OP.add,
                )

                yield "l3"
                # union softmax: ua = s3 / sum(w3 * raT)
                mx3 = tmp.tile([P, 4, O], F32, tag=f"mx3{h}")
                nc.vector.reduce_max(mx3, l3T[:, :, osl, :], axis=AX)
                d3 = tmp.tile([P, 4, O, 10], F32, tag=f"d3{h}")
                nc.vector.tensor_tensor(
                    out=d3,
                    in0=l3T[:, :, osl, :],
                    in1=mx3.broadcast_to([P, 4, O, 10]),
                    op=OP.subtract,
                )
                w3 = tmp.tile([P, 4, O, 10], F32, tag=f"w3{h}")
                nc.scalar.activation(out=w3, in_=d3, func=AF.Exp)
                s3 = tmp.tile([P, 4, O], F32, tag=f"s3{h}")
                nc.vector.reduce_sum(s3, w3, axis=AX)
                tua = tmp.tile([P, 4, O, 10], F32, tag=f"tua{h}")
                nc.vector.tensor_tensor(
                    out=tua, in0=w3, in1=raT[:, :, osl, :], op=OP.mult
                )
                sua = tmp.tile([P, 4, O], F32, tag=f"sua{h}")
                nc.vector.reduce_sum(sua, tua, axis=AX)
                tub = tmp.tile([P, 4, O, 10], F32, tag=f"tub{h}")
                nc.gpsimd.tensor_tensor(
                    out=tub, in0=w3, in1=rbT[:, :, osl, :], op=OP.mult
                )
                sub = tmp.tile([P, 4, O], F32, tag=f"sub{h}")
                nc.vector.reduce_sum(sub, tub, axis=AX)
                rsa = tmp.tile([P, 4, O], F32, tag=f"rsa{h}")
                nc.vector.reciprocal(out=rsa, in_=sua)
                rsb = tmp.tile([P, 4, O], F32, tag=f"rsb{h}")
                nc.vector.reciprocal(out=rsb, in_=sub)
                nc.vector.tensor_tensor(
                    out=catFu[:, 0:4, osl], in0=s3, in1=rsa, op=OP.mult
                )
                nc.gpsimd.tensor_tensor(
                    out=catFu[:, 4:8, osl], in0=s3, in1=rsb, op=OP.mult
                )
                nc.vector.tensor_copy(
                    out=catF16[:, 0:8, osl], in_=catFu[:, :, osl]
                )

            for _ in chain(0):
                pass

            # ---- classify head chunks 8..32: emitted here so they stream
            # during the union-softmax vector phase (PE stays busy; the
            # accumulation chain stays open in a dedicated PSUM bank).
            # Chunk 32 (ones lhsT column x bl0/128 weights) adds bl0.
            # pf reuses the pseg banks (ps_all is dead after the x copy)
            pf = pseg.tile([O, 512], F32, tag="pf")
            for i, kc in enumerate(list(range(8, 33))):
                nc.tensor.matmul(
                    out=pf,
                    lhsT=catF16[:, kc, :],
                    rhs=wl0_sb[:, kc, :],
                    start=(i == 0),
                    stop=False,
                )

            # head chunks 0..7 close the accumulation chain
            for i, kc in enumerate(range(8)):
                nc.tensor.matmul(
                    out=pf,
                    lhsT=catF16[:, kc, :],
                    rhs=wl0_sb[:, kc, :],
                    start=False,
                    stop=(i == 7),
                )

            # out = relu(hf) . Wl + bl (bl0 added in the matmul chain);
            # halves split across scalar/gpsimd then vector/gpsimd.
            z_sb = rowsp.tile([O, 512], F32, tag="z_sb")
            nc.scalar.activation(
                out=z_sb[:, 0:256], in_=pf[:, 0:256], func=AF.Relu
            )
            nc.vector.tensor_scalar_max(
                out=z_sb[:, 256:512], in0=pf[:, 256:512], scalar1=0.0
            )
            hwv = rowsp.tile([O, 256], F32, tag="hwv")
            nc.vector.tensor_tensor(
                out=hwv, in0=z_sb[:, 0:256], in1=wlr_sb[:, 0:256], op=OP.mult
            )
            sva = rowsp.tile([O, 1], F32, tag="sva")
            nc.vector.reduce_sum(sva, hwv, axis=AX)
            hwg = rowsp.tile([O, 256], F32, tag="hwg")
            nc.gpsimd.tensor_tensor(
                out=hwg, in0=z_sb[:, 256:512], in1=wlr_sb[:, 256:512], op=OP.mult
            )
            svb = rowsp.tile([O, 1], F32, tag="svb")
            nc.vector.reduce_sum(svb, hwg, axis=AX)
            osum = rowsp.tile([O, 1], F32, tag="osum")
            nc.vector.tensor_tensor(out=osum, in0=sva, in1=svb, op=OP.add)
            out_sb = rowsp.tile([O, 1], F32, tag="out_sb")
            nc.vector.tensor_scalar_add(
                out=out_sb, in0=osum, scalar1=bias_sb[0:O, 20:21]
            )
            nc.sync.dma_start(out=out_d[:], in_=out_sb)

            if debug:
                for name, t in (
                    ("xT", xT),
                    ("eT16", eT16),
                    ("l1T", l1T),
                    ("cat2", cat2),
                    ("raT", raT),
                    ("rbT", rbT),
                    ("catF16", catF16),
                ):
                    dt = t.dtype if hasattr(t, "dtype") else F32
                    d_ = nc.dram_tensor(
                        "dbg_" + name, list(t.shape), dt, kind="ExternalOutput"
                    )
                    nc.sync.dma_start(out=d_[:], in_=t)

    _split_excess_waits(nc)
    return nc


_NC = None


def _get_nc():
    global _NC
    if _NC is None:
        _NC = _build_nc()
    return _NC


def _prep_inputs(hidden, idx, Wp, bp, Wa0, ba0, Wa, ba, Wl0, bl0, Wl, bl):
    hidden = np.asarray(hidden, dtype=np.float32)
    idx = np.asarray(idx).astype(np.int64)

    f32 = lambda a: np.ascontiguousarray(np.asarray(a, dtype=np.float32))
    f16 = lambda a: np.ascontiguousarray(
        np.asarray(a, dtype=np.float32).astype(NPF16)
    )
    bp, ba0, ba, bl0, bl = f32(bp), f32(ba0), f32(ba), f32(bl0), f32(bl)
    Wl = f32(Wl)

    hid_f8 = np.ascontiguousarray(hidden.astype(NPF8))  # [B, O, L, E]
    f8 = lambda a: np.ascontiguousarray(
        np.asarray(a, dtype=np.float32).astype(NPF8)
    )
    wp_t = f8(np.asarray(Wp, np.float32).reshape(8, P, 1024).transpose(1, 0, 2))
    wa0_t = f8(np.asarray(Wa0, np.float32).reshape(8, P, 512).transpose(1, 0, 2))
    wa_t = f8(np.asarray(Wa, np.float32).reshape(4, P, 512).transpose(1, 0, 2))
    wl0_ext = np.concatenate(
        [
            np.asarray(Wl0, np.float32).reshape(32, P, 512),
            np.broadcast_to(bl0 / P, (1, P, 512)).astype(np.float32),
        ],
        axis=0,
    )
    wl0_t = f16(wl0_ext.transpose(1, 0, 2))
    ident = np.ascontiguousarray(np.eye(P, dtype=np.float32).astype(NPF16))

    biases = np.zeros((P, 21), dtype=np.float32)
    biases[:, 0:8] = (bp + 1.0).reshape(8, P).T
    biases[:, 8:12] = ba0.reshape(4, P).T
    biases[:, 12:16] = ba.reshape(4, P).T
    biases[:, 16:20] = bl0.reshape(4, P).T
    biases[:, 20] = bl[0]

    wlrep = np.ascontiguousarray(
        np.broadcast_to(Wl[:, 0], (O, 512)).astype(np.float32)
    )

    in_maps = []
    for b in range(B):
        m = np.zeros((L, NK), dtype=np.float32)
        cntinv = np.zeros((NK,), dtype=np.float32)
        ib = idx[b]
        starts = [1] + [int(ib[k]) for k in range(9)]
        ends = [int(ib[k]) for k in range(10)]
        segs = [(starts[k], ends[k]) for k in range(10)]
        segs.append((int(ib[9]), int(ib[10])))
        segs.append((int(ib[10]), int(ib[11])))
        segs.append((1, int(ib[9])))
        for k, (s, e) in enumerate(segs):
            m[s:e, k] = 1.0
            cntinv[k] = 1.0 / (e - s)
        # p-outer: maskt[p, t, k] pairs with hidden rows l = p*T + t
        maskt = np.ascontiguousarray(m.reshape(P, T, NK).astype(NPF8))
        dmat = np.zeros((P, NCOL), dtype=np.float32)
        for o in range(O):
            for k in range(NK):
                dmat[o * 32 + k, o * NK + k] = cntinv[k]
        dmat = np.ascontiguousarray(dmat.astype(NPF16))

        in_maps.append(
            dict(
                hidden=np.ascontiguousarray(hid_f8[b]),
                maskt=maskt,
                dmat=dmat,
                ident=ident,
                wp=wp_t,
                wa0=wa0_t,
                wa=wa_t,
                wl0=wl0_t,
                biases=biases,
                wlrep=wlrep,
            )
        )
    return in_maps


def _run(in_maps, **kwargs):
    return run_bass_kernel_spmd(_get_nc(), in_maps, core_ids=list(range(B)), **kwargs)


def kernel(**inputs):
    in_maps = _prep_inputs(**inputs)
    res = _run(in_maps)
    return np.stack([r["out"].reshape(O, 1) for r in res.results])


def _install_ntff_hook():
    """The RL container's antenv lacks axon_hooks, so boot() skipped NTFF
    hook registration. Recreate the module and register the ctypes hook."""
    import sys
    import types

    name = "antenv.axon_hooks"
    if name not in sys.modules:
        try:
            __import__(name)
        except ImportError:
            mod = types.ModuleType(name)
            mod._hook = None
            mod.set_axon_ntff_profile_hook = lambda h: setattr(mod, "_hook", h)
            mod.get_axon_ntff_profile_hook = lambda: mod._hook
            sys.modules[name] = mod
            import antenv

            antenv.axon_hooks = mod
    import antenv.axon_hooks as ah

    if ah.get_axon_ntff_profile_hook() is None:
        from trn_agent_boot.trn_boot import _ntff_profile_via_ctypes

        ah.set_axon_ntff_profile_hook(
            _ntff_profile_via_ctypes("/opt/axon/libaxon_pjrt.so")
        )

    import concourse.bass_utils as bu

    bu.upload_artifacts = lambda tmpdir: tmpdir


def benchmark(trace_cores=None, **inputs):
    """Run with NTFF tracing; returns (output, BassKernelResults)."""
    _install_ntff_hook()
    in_maps = _prep_inputs(**inputs)
    res = _run(in_maps, trace=True, trace_cores=trace_cores)
    out = np.stack([r["out"].reshape(O, 1) for r in res.results])
    return out, res


# revision 45
# speedup vs baseline: 1.2308x; 1.2308x over previous
"""Trainium2 Bass kernel for nn_Beta_LR_41308995453190.

Network (per (b, o) pair):
  - 13 segment means over the L axis of hidden[b, o] (ragged boundaries
    from idx[b]): 10 context segments, question, option, whole-context.
  - beta-param projection e = 1 + relu(x @ Wp + bp), split a/b.
  - three attention pools (intersection over segments, renew over
    (segment, intersection) pairs, union over inverted renewed params).
  - classify head: concat 8 beta embeddings -> relu(@Wl0 + bl0) -> @Wl + bl.

Sharding: data-parallel over the batch dim B=8 (one batch per NeuronCore),
weights replicated.

Implementation notes:
  - hidden + segment masks + Wp/Wa0/Wa travel in fp8 e4m3 (measured
    end-to-end error ~3.3e-3 vs the 2e-2 gate); Wl0 stays fp16 (fp8
    there costs 2.9e-2). Activations and everything else 16-bit is fp16.
  - hidden uses a p-outer SBUF layout (partition p holds L rows
    [p*T, (p+1)*T)) so every DMA descriptor is an 8 KB contiguous run;
    option-0 hidden is the first DMA so the serial chain starts ASAP,
    small constants and weights queue behind the hidden options.
  - segment sums: per-option 0/1-mask matmuls (13-column fp8 stationary)
    packed into one [128, E] PSUM tile via PE column tile positions
    {0, 32, 64, 96}; a host-built selection matrix D (D[o*32+k, o*13+k]
    = 1/cnt_k) then transposes to feature-major while absorbing the
    1/count scaling and the 32->13 packing.
  - All layer matmuls run "flipped": the small activation block is the
    stationary operand, the weight matrix streams 512 columns at a time;
    outputs transpose back to feature-major (fp16 transposes) and the
    per-chunk biases apply in one batched vector op per layer.
  - The pair ("renew") softmax is computed as an exp blend
    1/na = (ea+1)*recip(ea*a+ia), ea = exp(l1-l2), keeping the scalar
    activation table on Exp (a Sigmoid would force two table reloads);
    the b-half elementwise work runs on gpsimd in parallel, and the
    reciprocals run per 2-chunk slice so the h3 matmuls pipeline behind
    them.
  - classify head: 33 chunks (chunk 32 = ones x bl0/128 applies the
    bias); wl0 DMA-groups and the 8..32 matmuls are ordered so they
    stream during the softmax vector phases; chunks 0..7 close the
    accumulation chain after the union.
"""

import numpy as np
import ml_dtypes

try:
    import concourse.bass as bass
except ImportError:
    import sys

    sys.path.insert(0, "/opt/trn_rl_repo")
    import concourse.bass as bass

import concourse.tile as tile
from concourse import mybir
from concourse.bass_utils import run_bass_kernel_spmd

F32 = mybir.dt.float32
F16 = mybir.dt.float16
FP8 = mybir.dt.float8e4
NPF16 = np.float16
NPF8 = ml_dtypes.float8_e4m3
AX = mybir.AxisListType.X
OP = mybir.AluOpType
AF = mybir.ActivationFunctionType

B, O, L, E = 8, 4, 1024, 1024
BETA = 512
NSEG = 12
NK = 13  # 10 ctx + q + o + allc
P = 128
T = L // P  # 8 L-tiles per option
NCOL = O * NK  # 52


# ---------------------------------------------------------------------------
# Workaround: this neuronxcc walrus build rejects more than one sem wait per
# TPB instruction ("Too many sync wait commands"). Hoist excess waits onto
# nop instructions inserted immediately before the offending instruction on
# the same engine.
# ---------------------------------------------------------------------------
def _split_excess_waits(nc, max_waits=1):
    scratch_bb = nc.cur_bb.bb
    for f in nc.m.functions:
        for bb in f.blocks:
            new_list = []
            for ins in bb.instructions:
                si = ins.sync_info
                waits = list(si.on_wait) if si and si.on_wait else []
                if len(waits) > max_waits:
                    for w in waits[: len(waits) - max_waits]:
                        carrier = nc.engines[ins.engine].nop(nofuse=True).ins
                        scratch_bb.instructions.remove(carrier)
                        carrier.sync_info = mybir.SyncInfo(
                            on_wait=[w], on_update=[]
                        )
                        new_list.append(carrier)
                    si.on_wait = waits[len(waits) - max_waits :]
                new_list.append(ins)
            bb.instructions[:] = new_list


def _patch_minimal_drain():
    """One-shot NEFF: skip the semaphore-clear + second all-engine barrier
    of the TileContext epilogue (they only matter when the program loops)."""
    from concourse.vector_clock import ScopedClock
    import concourse.tile as _tile

    def _drain_and_barrier(self, tick_clock, wait_clock):
        drain_inst = self.nc.sync.drain()
        wait_clock.add_sem_waits(
            drain_inst.ins, ScopedClock({None: tick_clock.global_clock})
        )
        self.nc.all_engine_barrier()
        assert self.sems is not None
        popped = self.nc._tile_sem_poison_stack.pop()
        assert popped is self._sem_poison

    _tile.TileContext._drain_and_barrier = _drain_and_barrier


_patch_minimal_drain()


def _build_nc(debug=False):
    nc = bass.Bass("TRN2", target_bir_lowering=False)

    hid_d = nc.dram_tensor("hidden", [O, L, E], FP8, kind="ExternalInput")
    mask_d = nc.dram_tensor("maskt", [P, T, NK], FP8, kind="ExternalInput")
    dmat_d = nc.dram_tensor("dmat", [P, NCOL], F16, kind="ExternalInput")
    ident_d = nc.dram_tensor("ident", [P, P], F16, kind="ExternalInput")
    wp_d = nc.dram_tensor("wp", [P, 8, 1024], FP8, kind="ExternalInput")
    wa0_d = nc.dram_tensor("wa0", [P, 8, 512], FP8, kind="ExternalInput")
    wa_d = nc.dram_tensor("wa", [P, 4, 512], FP8, kind="ExternalInput")
    # 33 chunks: chunk 32 is bl0/128 replicated, matched by an all-ones lhsT
    # column so the head matmul chain also applies the bl0 bias
    wl0_d = nc.dram_tensor("wl0", [P, 33, 512], F16, kind="ExternalInput")
    bias_d = nc.dram_tensor("biases", [P, 21], F32, kind="ExternalInput")
    wlr_d = nc.dram_tensor("wlrep", [O, 512], F32, kind="ExternalInput")
    out_d = nc.dram_tensor("out", [O, 1], F32, kind="ExternalOutput")

    with tile.TileContext(nc) as tc:
        with (
            tc.tile_pool(name="const", bufs=1) as const,
            tc.tile_pool(name="hidp2", bufs=4) as hidp2,
            tc.tile_pool(name="act", bufs=1) as act,
            tc.tile_pool(name="tmp", bufs=3) as tmp,
            tc.tile_pool(name="rows", bufs=1) as rowsp,
            tc.tile_pool(name="pseg", bufs=1, space="PSUM") as pseg,
            tc.tile_pool(name="prow", bufs=2, space="PSUM") as prow,
            tc.tile_pool(name="pt", bufs=2, space="PSUM") as pt,
        ):
            # ---- option-0 hidden goes absolutely first: the whole serial
            # chain (seg -> x -> e -> ... -> head) keys off hidden arrival
            htile0 = hidp2.tile([P, T, E], FP8, tag="htile")
            hid_r = hid_d.rearrange("o (p t) e -> o p t e", t=T)
            for q in range(4):
                nc.sync.dma_start(
                    out=htile0[:, q * 2 : q * 2 + 2, :],
                    in_=hid_r[0][:, q * 2 : q * 2 + 2, :],
                )

            # ---- mask first (needed by the first seg matmul); the other
            # small constants queue after the hidden options so the hidden
            # descriptor writes aren't delayed
            mask_sb = const.tile([P, T, NK], FP8)
            nc.sync.dma_start(out=mask_sb, in_=mask_d[:])
            dmat_sb = const.tile([P, NCOL], F16)
            ident = const.tile([P, P], F16)
            bias_sb = const.tile([P, 21], F32)
            wlr_sb = const.tile([O, 512], F32)

            # ---- packed segment sums: ps_all[o*32+k, e], one 13-column
            # fp8 mask stationary per option at PE column position o*32, so
            # each option's sums touch only its own partitions (this keeps
            # the two option-pair chains below independent).
            ps_all = pseg.tile([P, E], F32)
            nc.vector.memset(ps_all, 0.0)

            wp_sb = wa0_sb = wa_sb = wl0_sb = None
            for o in range(O):
                if o == 0:
                    htile = htile0
                else:
                    htile = hidp2.tile([P, T, E], FP8, tag="htile")
                    for q in range(4):
                        nc.sync.dma_start(
                            out=htile[:, q * 2 : q * 2 + 2, :],
                            in_=hid_r[o][:, q * 2 : q * 2 + 2, :],
                        )
                if o == O - 1:
                    # remaining small constants, then weights (fp8 except
                    # wl0); wl0 chunk groups ordered 8..32 first (those
                    # head matmuls only need eT), 0..7 (union) last
                    nc.sync.dma_start(out=dmat_sb, in_=dmat_d[:])
                    nc.sync.dma_start(out=ident, in_=ident_d[:])
                    nc.sync.dma_start(out=bias_sb, in_=bias_d[:])
                    nc.sync.dma_start(out=wlr_sb, in_=wlr_d[:])
                    # preload the scalar activation table off the critical
                    # path (the first scalar op otherwise loads it mid-kernel)
                    actwarm = const.tile([1, 1], F32)
                    nc.scalar.activation(
                        out=actwarm, in_=bias_sb[0:1, 0:1], func=AF.Exp
                    )
                    wp_sb = const.tile([P, 8, 1024], FP8)
                    for q in range(4):
                        nc.sync.dma_start(
                            out=wp_sb[:, q * 2 : q * 2 + 2, :],
                            in_=wp_d[:, q * 2 : q * 2 + 2, :],
                        )
                    wa0_sb = const.tile([P, 8, 512], FP8)
                    for q in range(2):
                        nc.sync.dma_start(
                            out=wa0_sb[:, q * 4 : q * 4 + 4, :],
                            in_=wa0_d[:, q * 4 : q * 4 + 4, :],
                        )
                    wa_sb = const.tile([P, 4, 512], FP8)
                    nc.sync.dma_start(out=wa_sb, in_=wa_d[:])
                    wl0_sb = const.tile([P, 33, 512], F16)
                    for g, w in ((8, 4), (12, 4), (16, 4), (20, 4), (24, 4),
                                 (28, 5), (0, 4), (4, 4)):
                        nc.sync.dma_start(
                            out=wl0_sb[:, g : g + w, :],
                            in_=wl0_d[:, g : g + w, :],
                        )
                for t in range(T):
                    for half in range(2):
                        sl = slice(half * 512, half * 512 + 512)
                        nc.tensor.matmul(
                            out=ps_all[o * 32 : o * 32 + NK, sl],
                            lhsT=mask_sb[:, t, :],
                            rhs=htile[:, t, sl],
                            start=(t == 0),
                            stop=(t == T - 1),
                            tile_position=(0, o * 32),
                        )

            # ---- the rest of the network runs as two independent
            # option-pair chains: chain 0 (options 0,1) starts as soon as
            # their hidden arrives, overlapping options 2,3's DMA; chain 1's
            # matmuls then fill chain 0's vector-phase PE gaps.
            x_sb = rowsp.tile([P, E], F16, tag="x_sb")
            xT = act.tile([P, 8, O, NK], F16)
            eT16 = act.tile([P, 8, O, NK], F16)
            catF16 = act.tile([P, 33, O], F16)
            nc.gpsimd.memset(catF16[:, 32, :], 1.0)
            cat2 = act.tile([P, 8, O], F32)
            cat2b = act.tile([P, 8, O], F16)
            l1T = act.tile([P, 4, O, NK], F32)
            l2T = act.tile([P, 4, O], F32)
            raT = act.tile([P, 4, O, 10], F16)
            rbT = act.tile([P, 4, O, 10], F16)
            l3T = act.tile([P, 4, O, 10], F32)
            catFu = act.tile([P, 8, O], F32)

            R = O * NK  # 52 rows

            def flip_layer(h, name, lhs_chunks, w_sb, n_out, r):
                """out rows = (lhs^T)^T @ W; returns the row-major fp16
                sbuf copy [r, n_out] for this chain."""
                rows_full = rowsp.tile([R, 1024], F16, tag=f"rows{h}")
                rows_sb = rows_full[:r, :n_out]
                for n2 in range(n_out // 512):
                    pr = prow.tile([r, 512], F32, tag="prow")
                    for c, lhs in enumerate(lhs_chunks):
                        nc.tensor.matmul(
                            out=pr,
                            lhsT=lhs,
                            rhs=w_sb[:, c, n2 * 512 : (n2 + 1) * 512]
                            if w_sb.shape[2] > 512
                            else w_sb[:, c, :],
                            start=(c == 0),
                            stop=(c == len(lhs_chunks) - 1),
                        )
                    b = n2 * 512
                    nc.scalar.copy(
                        out=rows_sb[:, b : b + 256], in_=pr[:, 0:256]
                    )
                    nc.vector.tensor_copy(
                        out=rows_sb[:, b + 256 : b + 512], in_=pr[:, 256:512]
                    )
                return rows_sb

            def transpose_all(rows_sb, r, n_out):
                tpt = pt.tile([P, n_out // P, r], F16, tag="ptT")
                for mc in range(n_out // P):
                    nc.tensor.transpose(
                        out=tpt[:, mc, :],
                        in_=rows_sb[:, mc * P : (mc + 1) * P],
                        identity=ident[:r, :r],
                    )
                return tpt

            def chain(h):
                osl = slice(0, O)
                psl = slice(0, P)
                # x: fp16 copy of the packed sums per 128-column chunk
                # (alternating engines) so each scaled D-transpose starts as
                # soon as its chunk lands
                for c in range(8):
                    cs = slice(c * P, (c + 1) * P)
                    if c % 2 == 0:
                        nc.scalar.copy(out=x_sb[:, cs], in_=ps_all[:, cs])
                    else:
                        nc.vector.tensor_copy(out=x_sb[:, cs], in_=ps_all[:, cs])
                for c in range(8):
                    xtp = prow.tile([P, R], F32, tag="prow")
                    nc.tensor.matmul(
                        out=xtp,
                        lhsT=x_sb[:, c * P : (c + 1) * P],
                        rhs=dmat_sb,
                        start=True,
                        stop=True,
                    )
                    if c % 2 == 0:
                        nc.scalar.copy(out=xT[:, c, osl, :], in_=xtp)
                    else:
                        nc.vector.tensor_copy(out=xT[:, c, osl, :], in_=xtp)

                yield "x"
                # projection e = max(x @ Wp + (bp + 1), 1)
                rows_e = flip_layer(
                    h, "e", [xT[:, c, osl, :] for c in range(8)], wp_sb,
                    1024, R,
                )
                tp_e = transpose_all(rows_e, R, 1024)
                e_flat = eT16[:, :, osl, :].rearrange("p m o k -> p m (o k)")
                nc.vector.tensor_tensor(
                    out=e_flat,
                    in0=tp_e,
                    in1=bias_sb[:, 0:8].broadcast_to([P, 8, R]),
                    op=OP.add,
                )
                nc.vector.tensor_scalar_max(out=e_flat, in0=e_flat, scalar1=1.0)

                # catF chunks 8..31 only need eT
                for j, (ab, k) in enumerate(
                    ((0, 12), (1, 12), (0, 11), (1, 11), (0, 10), (1, 10))
                ):
                    nc.gpsimd.tensor_copy(
                        out=catF16[:, 8 + j * 4 : 12 + j * 4, osl],
                        in_=eT16[:, ab * 4 : ab * 4 + 4, osl, k],
                    )

                yield "e"
                # pool 1 (intersection): h1 = relu(e @ Wa0 + ba0)
                h1Tb = act.tile([P, 4, R], F16, tag=f"h1T{h}")
                rows_h1 = flip_layer(
                    h, "h1", [eT16[:, c, osl, :] for c in range(8)], wa0_sb,
                    512, R,
                )
                tp_h1 = transpose_all(rows_h1, R, 512)
                nc.vector.tensor_tensor(
                    out=h1Tb,
                    in0=tp_h1,
                    in1=bias_sb[:, 8:12].broadcast_to([P, 4, R]),
                    op=OP.add,
                )
                nc.vector.tensor_scalar_max(out=h1Tb, in0=h1Tb, scalar1=0.0)

                yield "h1"
                # l1 = h1 @ Wa + ba
                rows_l1 = flip_layer(
                    h, "l1", [h1Tb[:, c, :] for c in range(4)], wa_sb, 512, R
                )
                tp_l1 = transpose_all(rows_l1, R, 512)
                nc.vector.tensor_tensor(
                    out=l1T[:, :, osl, :].rearrange("p m o k -> p m (o k)"),
                    in0=tp_l1,
                    in1=bias_sb[:, 12:16].broadcast_to([P, 4, R]),
                    op=OP.add,
                )

                yield "l1"
                # pool 1 softmax over the 10 ctx segments + weighted reduce
                lsl = l1T[:, :, osl, 0:10]
                mx = tmp.tile([P, 4, O], F32, tag=f"mx{h}")
                nc.vector.reduce_max(mx, lsl, axis=AX)
                d = tmp.tile([P, 4, O, 10], F32, tag=f"d{h}")
                nc.vector.tensor_tensor(
                    out=d, in0=lsl, in1=mx.broadcast_to([P, 4, O, 10]),
                    op=OP.subtract,
                )
                w = tmp.tile([P, 4, O, 10], F32, tag=f"w{h}")
                nc.scalar.activation(out=w, in_=d, func=AF.Exp)
                sums = tmp.tile([P, 4, O], F32, tag=f"s{h}")
                nc.vector.reduce_sum(sums, w, axis=AX)
                r_ = tmp.tile([P, 4, O], F32, tag=f"r{h}")
                nc.vector.reciprocal(out=r_, in_=sums)
                wa_t = tmp.tile([P, 4, O, 10], F32, tag=f"wa_t{h}")
                nc.vector.tensor_tensor(
                    out=wa_t, in0=w, in1=eT16[:, 0:4, osl, 0:10], op=OP.mult
                )
                sa1 = tmp.tile([P, 4, O], F32, tag=f"sa1{h}")
                nc.vector.reduce_sum(sa1, wa_t, axis=AX)
                nc.vector.tensor_tensor(
                    out=cat2[:, 0:4, osl], in0=sa1, in1=r_, op=OP.mult
                )
                wb_t = tmp.tile([P, 4, O, 10], F32, tag=f"wb_t{h}")
                nc.gpsimd.tensor_tensor(
                    out=wb_t, in0=w, in1=eT16[:, 4:8, osl, 0:10], op=OP.mult
                )
                sb1 = tmp.tile([P, 4, O], F32, tag=f"sb1{h}")
                nc.vector.reduce_sum(sb1, wb_t, axis=AX)
                nc.gpsimd.tensor_tensor(
                    out=cat2[:, 4:8, osl], in0=sb1, in1=r_, op=OP.mult
                )
                nc.vector.tensor_copy(out=cat2b[:, :, osl], in_=cat2[:, :, osl])

                yield "sm1"
                # renew: h2/l2 for the intersection pair element
                h2Tb = act.tile([P, 4, O], F16, tag=f"h2T{h}")
                rows_h2 = flip_layer(
                    h, "h2", [cat2b[:, c, osl] for c in range(8)], wa0_sb,
                    512, O,
                )
                tp_h2 = transpose_all(rows_h2, O, 512)
                nc.vector.tensor_tensor(
                    out=h2Tb,
                    in0=tp_h2,
                    in1=bias_sb[:, 8:12].broadcast_to([P, 4, O]),
                    op=OP.add,
                )
                nc.vector.tensor_scalar_max(out=h2Tb, in0=h2Tb, scalar1=0.0)
                rows_l2 = flip_layer(
                    h, "l2", [h2Tb[:, c, :] for c in range(4)], wa_sb, 512, O
                )
                tp_l2 = transpose_all(rows_l2, O, 512)
                nc.vector.tensor_tensor(
                    out=l2T[:, :, osl],
                    in0=tp_l2,
                    in1=bias_sb[:, 12:16].broadcast_to([P, 4, O]),
                    op=OP.add,
                )

                yield "h2l2"
                # pair softmax as an exp blend: 1/na = (ea+1)*recip(ea*a+ia)
                l1s = l1T[:, :, osl, 0:10]
                l2b = l2T[:, :, osl].broadcast_to([P, 4, O, 10])
                d12 = tmp.tile([P, 4, O, 10], F32, tag=f"d12{h}")
                nc.vector.tensor_tensor(out=d12, in0=l1s, in1=l2b, op=OP.subtract)
                ea = tmp.tile([P, 4, O, 10], F32, tag=f"ea{h}")
                nc.scalar.activation(out=ea, in_=d12, func=AF.Exp)
                sp1 = tmp.tile([P, 4, O, 10], F32, tag=f"sp1{h}")
                # scalar engine: sp1 = Copy(ea * 1 + 1); the Pool engine
                # takes 2.6us for this op and blocks the chain
                nc.scalar.activation(
                    out=sp1, in_=ea, func=AF.Copy, bias=1.0
                )
                nvs = []
                for ab, dst, eng in (
                    (0, raT, nc.vector),
                    (1, rbT, nc.gpsimd),
                ):
                    ia_b = cat2[:, ab * 4 : ab * 4 + 4, osl].broadcast_to(
                        [P, 4, O, 10]
                    )
                    t1 = tmp.tile([P, 4, O, 10], F32, tag=f"t1{ab}{h}")
                    eng.tensor_tensor(
                        out=t1,
                        in0=ea,
                        in1=eT16[:, ab * 4 : ab * 4 + 4, osl, 0:10],
                        op=OP.mult,
                    )
                    nv = tmp.tile([P, 4, O, 10], F32, tag=f"nv{ab}{h}")
                    # both nv adds on gpsimd: the same add on vector measures
                    # 1.4us (vs 0.5 here), and vector is the recip bottleneck
                    nc.gpsimd.tensor_tensor(out=nv, in0=t1, in1=ia_b, op=OP.add)
                    nvs.append((dst, nv))
                with nc.allow_low_precision("1/na feeds fp16 consumers"):
                    for dst, nv in nvs:
                        for c0 in (0, 2):
                            rc = tmp.tile([P, 2, O, 10], F32, tag=f"rc{c0}{h}")
                            nc.vector.reciprocal(
                                out=rc, in_=nv[:, c0 : c0 + 2, :, :]
                            )
                            nc.vector.tensor_tensor(
                                out=dst[:, c0 : c0 + 2, osl, :],
                                in0=rc,
                                in1=sp1[:, c0 : c0 + 2, :, :],
                                op=OP.mult,
                            )

                yield "pair"
                # union pool over segments of [1/na; 1/nb]
                h3Tb = act.tile([P, 4, O, 10], F16, tag=f"h3T{h}")
                rows_h3 = flip_layer(
                    h, "h3",
                    [raT[:, c, osl, :] for c in range(4)]
                    + [rbT[:, c, osl, :] for c in range(4)],
                    wa0_sb, 512, O * 10,
                )
                tp_h3 = transpose_all(rows_h3, O * 10, 512)
                h3_flat = h3Tb.rearrange("p m o k -> p m (o k)")
                nc.vector.tensor_tensor(
                    out=h3_flat,
                    in0=tp_h3,
                    in1=bias_sb[:, 8:12].broadcast_to([P, 4, O * 10]),
                    op=OP.add,
                )
                nc.vector.tensor_scalar_max(out=h3_flat, in0=h3_flat, scalar1=0.0)
                yield "h3"
                rows_l3 = flip_layer(
                    h, "l3", [h3Tb[:, c, :, :] for c in range(4)], wa_sb,
                    512, O * 10,
                )
                tp_l3 = transpose_all(rows_l3, O * 10, 512)
                nc.vector.tensor_tensor(
                    out=l3T[:, :, osl, :].rearrange("p m o k -> p m (o k)"),
                    in0=tp_l3,
                    in1=bias_sb[:, 12:16].broadcast_to([P, 4, O * 10]),
                    op=OP.add,
                )

                yield "l3"
                # union softmax: ua = s3 / sum(w3 * raT)
                mx3 = tmp.tile([P, 4, O], F32, tag=f"mx3{h}")
                nc.vector.reduce_max(mx3, l3T[:, :, osl, :], axis=AX)
                d3 = tmp.tile([P, 4, O, 10], F32, tag=f"d3{h}")
                nc.vector.tensor_tensor(
                    out=d3,
                    in0=l3T[:, :, osl, :],
                    in1=mx3.broadcast_to([P, 4, O, 10]),
                    op=OP.subtract,
                )
                w3 = tmp.tile([P, 4, O, 10], F32, tag=f"w3{h}")
                nc.scalar.activation(out=w3, in_=d3, func=AF.Exp)
                s3 = tmp.tile([P, 4, O], F32, tag=f"s3{h}")
                nc.vector.reduce_sum(s3, w3, axis=AX)
                tua = tmp.tile([P, 4, O, 10], F32, tag=f"tua{h}")
                nc.vector.tensor_tensor(
                    out=tua, in0=w3, in1=raT[:, :, osl, :], op=OP.mult
                )
                sua = tmp.tile([P, 4, O], F32, tag=f"sua{h}")
                nc.vector.reduce_sum(sua, tua, axis=AX)
                tub = tmp.tile([P, 4, O, 10], F32, tag=f"tub{h}")
                nc.gpsimd.tensor_tensor(
                    out=tub, in0=w3, in1=rbT[:, :, osl, :], op=OP.mult
                )
                sub = tmp.tile([P, 4, O], F32, tag=f"sub{h}")
                nc.vector.reduce_sum(sub, tub, axis=AX)
                rsa = tmp.tile([P, 4, O], F32, tag=f"rsa{h}")
                nc.vector.reciprocal(out=rsa, in_=sua)
                rsb = tmp.tile([P, 4, O], F32, tag=f"rsb{h}")
                nc.vector.reciprocal(out=rsb, in_=sub)
                nc.vector.tensor_tensor(
                    out=catFu[:, 0:4, osl], in0=s3, in1=rsa, op=OP.mult
                )
                nc.gpsimd.tensor_tensor(
                    out=catFu[:, 4:8, osl], in0=s3, in1=rsb, op=OP.mult
                )
                nc.vector.tensor_copy(
                    out=catF16[:, 0:4, osl], in_=catFu[:, 0:4, osl]
                )
                nc.gpsimd.tensor_copy(
                    out=catF16[:, 4:8, osl], in_=catFu[:, 4:8, osl]
                )

            for _ in chain(0):
                pass

            # ---- classify head chunks 8..32: emitted here so they stream
            # during the union-softmax vector phase (PE stays busy; the
            # accumulation chain stays open in a dedicated PSUM bank).
            # Chunk 32 (ones lhsT column x bl0/128 weights) adds bl0.
            # pf reuses the pseg banks (ps_all is dead after the x copy)
            pf = pseg.tile([O, 512], F32, tag="pf")
            for i, kc in enumerate(list(range(8, 33))):
                nc.tensor.matmul(
                    out=pf,
                    lhsT=catF16[:, kc, :],
                    rhs=wl0_sb[:, kc, :],
                    start=(i == 0),
                    stop=False,
                )

            # head chunks 0..7 close the accumulation chain
            for i, kc in enumerate(range(8)):
                nc.tensor.matmul(
                    out=pf,
                    lhsT=catF16[:, kc, :],
                    rhs=wl0_sb[:, kc, :],
                    start=False,
                    stop=(i == 7),
                )

            # out = relu(hf) . Wl + bl (bl0 added in the matmul chain);
            # halves split across scalar/gpsimd then vector/gpsimd.
            z_sb = rowsp.tile([O, 512], F32, tag="z_sb")
            nc.scalar.activation(
                out=z_sb[:, 0:256], in_=pf[:, 0:256], func=AF.Relu
            )
            nc.vector.tensor_scalar_max(
                out=z_sb[:, 256:512], in0=pf[:, 256:512], scalar1=0.0
            )
            hwv = rowsp.tile([O, 256], F32, tag="hwv")
            nc.vector.tensor_tensor(
                out=hwv, in0=z_sb[:, 0:256], in1=wlr_sb[:, 0:256], op=OP.mult
            )
            sva = rowsp.tile([O, 1], F32, tag="sva")
            nc.vector.reduce_sum(sva, hwv, axis=AX)
            hwg = rowsp.tile([O, 256], F32, tag="hwg")
            nc.gpsimd.tensor_tensor(
                out=hwg, in0=z_sb[:, 256:512], in1=wlr_sb[:, 256:512], op=OP.mult
            )
            svb = rowsp.tile([O, 1], F32, tag="svb")
            nc.vector.reduce_sum(svb, hwg, axis=AX)
            osum = rowsp.tile([O, 1], F32, tag="osum")
            nc.vector.tensor_tensor(out=osum, in0=sva, in1=svb, op=OP.add)
            out_sb = rowsp.tile([O, 1], F32, tag="out_sb")
            nc.vector.tensor_scalar_add(
                out=out_sb, in0=osum, scalar1=bias_sb[0:O, 20:21]
            )
            nc.sync.dma_start(out=out_d[:], in_=out_sb)

            if debug:
                for name, t in (
                    ("xT", xT),
                    ("eT16", eT16),
                    ("l1T", l1T),
                    ("cat2", cat2),
                    ("raT", raT),
                    ("rbT", rbT),
                    ("catF16", catF16),
                ):
                    dt = t.dtype if hasattr(t, "dtype") else F32
                    d_ = nc.dram_tensor(
                        "dbg_" + name, list(t.shape), dt, kind="ExternalOutput"
                    )
                    nc.sync.dma_start(out=d_[:], in_=t)

    _split_excess_waits(nc)
    return nc


_NC = None


def _get_nc():
    global _NC
    if _NC is None:
        _NC = _build_nc()
    return _NC


def _prep_inputs(hidden, idx, Wp, bp, Wa0, ba0, Wa, ba, Wl0, bl0, Wl, bl):
    hidden = np.asarray(hidden, dtype=np.float32)
    idx = np.asarray(idx).astype(np.int64)

    f32 = lambda a: np.ascontiguousarray(np.asarray(a, dtype=np.float32))
    f16 = lambda a: np.ascontiguousarray(
        np.asarray(a, dtype=np.float32).astype(NPF16)
    )
    bp, ba0, ba, bl0, bl = f32(bp), f32(ba0), f32(ba), f32(bl0), f32(bl)
    Wl = f32(Wl)

    hid_f8 = np.ascontiguousarray(hidden.astype(NPF8))  # [B, O, L, E]
    f8 = lambda a: np.ascontiguousarray(
        np.asarray(a, dtype=np.float32).astype(NPF8)
    )
    wp_t = f8(np.asarray(Wp, np.float32).reshape(8, P, 1024).transpose(1, 0, 2))
    wa0_t = f8(np.asarray(Wa0, np.float32).reshape(8, P, 512).transpose(1, 0, 2))
    wa_t = f8(np.asarray(Wa, np.float32).reshape(4, P, 512).transpose(1, 0, 2))
    wl0_ext = np.concatenate(
        [
            np.asarray(Wl0, np.float32).reshape(32, P, 512),
            np.broadcast_to(bl0 / P, (1, P, 512)).astype(np.float32),
        ],
        axis=0,
    )
    wl0_t = f16(wl0_ext.transpose(1, 0, 2))
    ident = np.ascontiguousarray(np.eye(P, dtype=np.float32).astype(NPF16))

    biases = np.zeros((P, 21), dtype=np.float32)
    biases[:, 0:8] = (bp + 1.0).reshape(8, P).T
    biases[:, 8:12] = ba0.reshape(4, P).T
    biases[:, 12:16] = ba.reshape(4, P).T
    biases[:, 16:20] = bl0.reshape(4, P).T
    biases[:, 20] = bl[0]

    wlrep = np.ascontiguousarray(
        np.broadcast_to(Wl[:, 0], (O, 512)).astype(np.float32)
    )

    in_maps = []
    for b in range(B):
        m = np.zeros((L, NK), dtype=np.float32)
        cntinv = np.zeros((NK,), dtype=np.float32)
        ib = idx[b]
        starts = [1] + [int(ib[k]) for k in range(9)]
        ends = [int(ib[k]) for k in range(10)]
        segs = [(starts[k], ends[k]) for k in range(10)]
        segs.append((int(ib[9]), int(ib[10])))
        segs.append((int(ib[10]), int(ib[11])))
        segs.append((1, int(ib[9])))
        for k, (s, e) in enumerate(segs):
            m[s:e, k] = 1.0
            cntinv[k] = 1.0 / (e - s)
        # p-outer: maskt[p, t, k] pairs with hidden rows l = p*T + t
        maskt = np.ascontiguousarray(m.reshape(P, T, NK).astype(NPF8))
        dmat = np.zeros((P, NCOL), dtype=np.float32)
        for o in range(O):
            for k in range(NK):
                dmat[o * 32 + k, o * NK + k] = cntinv[k]
        dmat = np.ascontiguousarray(dmat.astype(NPF16))

        in_maps.append(
            dict(
                hidden=np.ascontiguousarray(hid_f8[b]),
                maskt=maskt,
                dmat=dmat,
                ident=ident,
                wp=wp_t,
                wa0=wa0_t,
                wa=wa_t,
                wl0=wl0_t,
                biases=biases,
                wlrep=wlrep,
            )
        )
    return in_maps


def _run(in_maps, **kwargs):
    return run_bass_kernel_spmd(_get_nc(), in_maps, core_ids=list(range(B)), **kwargs)


def kernel(**inputs):
    in_maps = _prep_inputs(**inputs)
    res = _run(in_maps)
    return np.stack([r["out"].reshape(O, 1) for r in res.results])


def _install_ntff_hook():
    """The RL container's antenv lacks axon_hooks, so boot() skipped NTFF
    hook registration. Recreate the module and register the ctypes hook."""
    import sys
    import types

    name = "antenv.axon_hooks"
    if name not in sys.modules:
        try:
            __import__(name)
        except ImportError:
            mod = types.ModuleType(name)
            mod._hook = None
            mod.set_axon_ntff_profile_hook = lambda h: setattr(mod, "_hook", h)
            mod.get_axon_ntff_profile_hook = lambda: mod._hook
            sys.modules[name] = mod
            import antenv

            antenv.axon_hooks = mod
    import antenv.axon_hooks as ah

    if ah.get_axon_ntff_profile_hook() is None:
        from trn_agent_boot.trn_boot import _ntff_profile_via_ctypes

        ah.set_axon_ntff_profile_hook(
            _ntff_profile_via_ctypes("/opt/axon/libaxon_pjrt.so")
        )

    import concourse.bass_utils as bu

    bu.upload_artifacts = lambda tmpdir: tmpdir


def benchmark(trace_cores=None, **inputs):
    """Run with NTFF tracing; returns (output, BassKernelResults)."""
    _install_ntff_hook()
    in_maps = _prep_inputs(**inputs)
    res = _run(in_maps, trace=True, trace_cores=trace_cores)
    out = np.stack([r["out"].reshape(O, 1) for r in res.results])
    return out, res
